# revision 9
# baseline (speedup 1.0000x reference)
"""Trainium2 Bass kernel for nn_AggregationRebuild_HN (sparse_attention).

Computes, for each of B=512 samples:
    out[b] = sum_j softmax(sim[b] / 0.02)[j] * block_j(b)          # [64, 128]
where block_j(b) are 3 "positive" rows (512 + 3b + j of p_enc_out) and 16
gathered "negative" rows (p_enc_out[negative_index[b, j]]).

Strategy ("pruned scatter-softmax-matmul", single-pass bf16):
  * Shard the P*D = 8192 feature axis across 8 cores (1024 features each).
  * At temperature 0.02 the softmax is extremely peaked: per sample only a
    handful of the 19 candidates carry weight >= 1e-9 of the max.  The host
    keeps exactly those (sample, candidate) pairs — duplicates within a
    sample first merged in log domain (logaddexp), so the device-side exp
    reproduces the summed softmax numerators — and packs them densely into
    K-slots of 128-tall chunks (typically 2 chunks per 128-sample tile).
    Dropped slots contribute < ~1e-8 relative error; the softmax
    DENOMINATOR is still computed on device from all 19 logits, exactly.
  * Each core ships only the pool rows referenced by live slots (host
    fancy-index gather, bf16): ~2.1 MiB instead of 16.8 MiB fp32 dense.
    Input DMAs alternate between the SP and ACT HWDGE rings, which map to
    disjoint 16-engine SDMA groups, roughly doubling load bandwidth.
  * The gather + weighted sum becomes short PE-matmul chains:
    out[tile, :] accumulates C matmuls [K=128 slots, M=128 samples,
    N=512 features] per feature half, in bf16 with fp32 PSUM accumulate.
  * WT (softmax numerators scattered into slot positions) is produced on
    device by ACT Exp over a host-built fp16 scatter of the max-shifted
    logits (empty slots hold -3e4 -> exp -> 0), cast straight to bf16, in
    per-tile pieces so the first matmuls start as soon as the first piece
    lands.  1/Z lands as a per-partition scale on the PSUM->SBUF drain,
    which writes bf16 in 256-col pieces (stores pipeline on the SP ring);
    the host upcasts on unshard.  End-to-end rel err ~2e-3, inside the
    2e-2 gate.
  * Dummy matmuls before and between the real chains hold the PE HAM
    activity window busy so the clock stays at 2.4 GHz (cold/idle default
    is 1.2 GHz, which also slows DMA and the drain engines).
  * Host-side work is index bookkeeping (pruning, scatter layout,
    log-domain duplicate merge, row gather), dtype casts, and the
    standard stable-softmax max shift only; exp/sum/normalize and all
    matvec math run on device.
"""

from contextlib import ExitStack

import numpy as np

_B = 512            # bs * n_vars
_P = 64             # patch_num
_D = 128            # d_model
_KP = 3             # k_positive
_KN = 16            # k_negative
_NCORES = 8
_PPC = _P // _NCORES        # patches per core = 8
_PDC = _PPC * _D            # features per core = 1024
_SENT = -3.0e4              # empty-slot sentinel; exp(50 * -3e4) == 0
_SCALE = 50.0               # 1 / temperature
_NTILES = _B // 128         # 4 M-tiles of 128 samples
_NLOGIT = _NTILES * (_KP + _KN)   # 76 raw-logit columns (for Z)
_WTHR = -20.72              # ln(1e-9): keep slots with weight >= 1e-9*max
_NWARM = 12                 # PE warm-up dummy matmuls


def _build_host(sim, neg_idx):
    """Prune + pack slots, build scatter and row list (vectorized).

    Returns (sscw, logits, rows, C):
      C: K-chunks of 128 slots per 128-sample tile (uniform, padded)
      sscw [128, 4*128*C] f16, sentinel in empty cells:
        slot s (chunk c = s>>7, partition p = s&127) of tile t, owned by
        sample b = 128t+m, holds its merged shifted logit at
        col 128*C*t + 128*c + m
      logits [128, 76] f32: raw shifted logits, [m, t*19+k] (for Z)
      rows [4*128*C]: p_enc_out row feeding each slot (0 for pad slots)
    """
    sim = np.asarray(sim, np.float32)
    neg_idx = np.asarray(neg_idx).astype(np.int64)
    simsh = (sim - sim.max(axis=1, keepdims=True)).astype(np.float32)  # [B, 19]

    # merge duplicate negative rows per sample in log domain
    acc = np.full((_B, _B), -np.inf)
    bb = np.repeat(np.arange(_B), _KN)
    np.logaddexp.at(
        acc, (bb, neg_idx.ravel()),
        (_SCALE * simsh[:, _KP:].astype(np.float64)).ravel(),
    )
    nb, nr = np.nonzero(acc >= _WTHR)            # relevant negatives
    nv = (acc[nb, nr] / _SCALE).astype(np.float32)

    pb, pj = np.nonzero(simsh[:, :_KP] >= _WTHR / _SCALE)  # relevant positives
    pv = simsh[pb, pj]
    pr = _B + _KP * pb + pj

    samp = np.concatenate([nb, pb])
    rows = np.concatenate([nr, pr])
    vals = np.concatenate([nv, pv])
    tile = samp >> 7
    order = np.argsort(tile, kind="stable")
    samp, rows, vals, tile = samp[order], rows[order], vals[order], tile[order]
    counts = np.bincount(tile, minlength=_NTILES)
    C = max(1, int(-(-counts.max() // 128)))
    starts = np.concatenate([[0], np.cumsum(counts)[:-1]])
    slot = np.arange(len(samp)) - starts[tile]   # slot index within tile

    nslot = 128 * C
    sscw = np.full((128, _NTILES * nslot), _SENT, np.float16)
    col = nslot * tile + 128 * (slot >> 7) + (samp & 127)
    sscw[slot & 127, col] = vals.astype(np.float16)
    logits = np.ascontiguousarray(
        simsh.reshape(_NTILES, 128, 19).transpose(1, 0, 2).reshape(128, -1)
    )

    row_list = np.zeros(_NTILES * nslot, np.int64)
    row_list[nslot * tile + slot] = rows
    return sscw, logits, row_list, C


def _kernel_body(ctx, tc, out_ap, pool_ap, sscw_ap, logit_ap, C):
    import concourse.mybir as mybir

    nc = tc.nc
    f32 = mybir.dt.float32
    f16 = mybir.dt.float16
    bf16 = mybir.dt.bfloat16
    AF = mybir.ActivationFunctionType
    nslot = 128 * C
    nsloc = _NTILES * nslot            # total slot columns

    const = ctx.enter_context(tc.tile_pool(name="const", bufs=1))
    psum_pool = ctx.enter_context(tc.tile_pool(name="psum", bufs=8, space="PSUM"))

    ps = {
        (t, h): psum_pool.tile(
            [128, 512], f32, tag=f"ps{t}{h}", name=f"ps{t}{h}", bufs=1
        )
        for t in range(_NTILES)
        for h in range(2)
    }

    # --- PE warm-up + HAM keepalive: dummy matmuls into ps[3,1], which the
    # real (3,1) chain's start=True reset wipes -----------------------------
    warm = const.tile([128, 512], bf16, tag="warm")
    nc.vector.memset(warm[:], 0.0)

    def dummy_mm(n):
        # short (N=128) matmuls: enough PE activity for the HAM window at
        # ~1/4 the cost of a full-width dummy
        for _ in range(n):
            nc.tensor.matmul(
                ps[3, 1][:, 0:128], lhsT=warm[:, 0:128], rhs=warm[:, 0:128],
                start=True, stop=True, skip_group_check=True,
            )

    dummy_mm(_NWARM)

    sscw = const.tile([128, nsloc], f16, tag="sscw")
    logit = const.tile([128, _NLOGIT], f32, tag="logit")
    whi = const.tile([128, nsloc], bf16, tag="whi")
    pool_sb = const.tile([128, _NTILES * C * _PDC], bf16, tag="pool")

    def load_tile(t):
        # pool on one HWDGE ring, scatter on the other (disjoint SDMA
        # engine groups); alternate by tile parity to balance bytes
        pool_eng = nc.sync if (t % 2 == 0) else nc.scalar
        ssc_eng = nc.scalar if (t % 2 == 0) else nc.sync
        c0 = nslot * t
        ssc_eng.dma_start(
            out=sscw[:, c0 : c0 + nslot], in_=sscw_ap[:, c0 : c0 + nslot]
        )
        view = pool_ap[nslot * t : nslot * (t + 1)].rearrange(
            "(c p) n -> c p n", p=128
        )
        o0 = C * _PDC * t
        pool_eng.dma_start(
            out=pool_sb[:, o0 : o0 + C * _PDC].rearrange("p (c n) -> p c n", n=_PDC),
            in_=view.rearrange("c p n -> p c n"),
        )

    def split_w(t):
        c0 = nslot * t
        nc.scalar.activation(
            out=whi[:, c0 : c0 + nslot], in_=sscw[:, c0 : c0 + nslot],
            func=AF.Exp, scale=_SCALE,
        )

    load_tile(0)
    nc.sync.dma_start(out=logit[:], in_=logit_ap[:])
    split_w(0)
    for t in range(1, _NTILES):
        load_tile(t)
        split_w(t)

    # softmax denominator Z[b] and 1/Z (exp + row-sum fused via accum_out)
    rz = const.tile([128, _NTILES], f32, tag="rz")
    for t in range(_NTILES):
        e = const.tile([128, 19], f32, tag=f"e{t}", name=f"e{t}")
        z = const.tile([128, 1], f32, tag=f"z{t}", name=f"z{t}")
        nc.scalar.activation(
            out=e[:],
            in_=logit[:, 19 * t : 19 * (t + 1)],
            func=AF.Exp,
            scale=_SCALE,
            accum_out=z[:],
        )
        nc.vector.reciprocal(out=rz[:, t : t + 1], in_=z[:])

    # --- matmul chains + per-tile drains (split in 256-col pieces so the
    # stores pipeline); drains alternate ACT / DVE; stores on the SP ring --
    out_sb = const.tile([128, _NTILES * _PDC], bf16, tag="out_sb")
    out_view = out_ap.rearrange("(t p) n -> t p n", p=128)
    for t in range(_NTILES):
        for h in range(2):
            for c in range(C):
                nc.tensor.matmul(
                    ps[t, h][:],
                    lhsT=whi[:, nslot * t + 128 * c : nslot * t + 128 * (c + 1)],
                    rhs=pool_sb[
                        :,
                        (C * t + c) * _PDC + 512 * h : (C * t + c) * _PDC + 512 * h + 512,
                    ],
                    start=(c == 0),
                    stop=(c == C - 1),
                    skip_group_check=True,
                )
        last = t == _NTILES - 1
        # the last tile's drains split in halves so its stores start sooner
        for h in range(2):
            for p0, p1 in ([(0, 256), (256, 512)] if last else [(0, 512)]):
                dst = out_sb[:, _PDC * t + 512 * h + p0 : _PDC * t + 512 * h + p1]
                if h == 0:
                    nc.scalar.activation(
                        out=dst, in_=ps[t, h][:, p0:p1],
                        func=AF.Copy, scale=rz[:, t : t + 1],
                    )
                    nc.scalar.dma_start(
                        out=out_view[t, :, 512 * h + p0 : 512 * h + p1], in_=dst
                    )
                else:
                    nc.vector.tensor_scalar_mul(
                        dst, ps[t, h][:, p0:p1], rz[:, t : t + 1]
                    )
                    nc.sync.dma_start(
                        out=out_view[t, :, 512 * h + p0 : 512 * h + p1], in_=dst
                    )
        if not last:
            dummy_mm(3)  # HAM keepalive while waiting for the next tile


_prog_cache = {}


def _get_program(C):
    if C in _prog_cache:
        return _prog_cache[C]
    import concourse.bacc as bacc
    import concourse.mybir as mybir
    import concourse.tile as tile

    nc = bacc.Bacc(
        "TRN2",
        target_bir_lowering=False,
        debug=False,
        enable_asserts=False,
        num_devices=_NCORES,
    )
    f32 = mybir.dt.float32
    f16 = mybir.dt.float16
    bf16 = mybir.dt.bfloat16
    pool_ap = nc.dram_tensor(
        "pool", [_NTILES * 128 * C, _PDC], bf16, kind="ExternalInput"
    ).ap()
    sscw_ap = nc.dram_tensor(
        "sscw", [128, _NTILES * 128 * C], f16, kind="ExternalInput"
    ).ap()
    logit_ap = nc.dram_tensor(
        "logit", [128, _NLOGIT], f32, kind="ExternalInput"
    ).ap()
    out_ap = nc.dram_tensor("out", [_B, _PDC], bf16, kind="ExternalOutput").ap()
    with tile.TileContext(nc) as tc:
        with ExitStack() as ctx:
            _kernel_body(ctx, tc, out_ap, pool_ap, sscw_ap, logit_ap, C)
    nc.compile()
    _prog_cache[C] = nc
    return nc


def _prepare(similarity_matrix, p_enc_out, negative_index):
    import ml_dtypes

    sim = np.asarray(similarity_matrix, np.float32)
    pool = np.asarray(p_enc_out, np.float32)
    assert sim.shape == (_B, _KP + _KN), sim.shape
    assert pool.shape == (_B * (1 + _KP), _P, _D), pool.shape
    sscw, logits, row_list, C = _build_host(sim, negative_index)
    poolb = pool.astype(ml_dtypes.bfloat16)      # [2048, 64, 128]
    gathered = poolb[row_list]                   # [4*128*C, 64, 128]
    in_maps = []
    for c in range(_NCORES):
        sl = gathered[:, _PPC * c : _PPC * (c + 1), :].reshape(-1, _PDC)
        in_maps.append(
            {"pool": np.ascontiguousarray(sl), "sscw": sscw, "logit": logits}
        )
    return in_maps, C


def _postprocess(results):
    outs = [
        r["out"].astype(np.float32).reshape(_B, _PPC, _D) for r in results
    ]
    return np.ascontiguousarray(np.concatenate(outs, axis=1))


def kernel(similarity_matrix, p_enc_out, negative_index, **_unused):
    from concourse.bass_utils import run_bass_kernel_spmd

    in_maps, C = _prepare(similarity_matrix, p_enc_out, negative_index)
    nc = _get_program(C)
    res = run_bass_kernel_spmd(nc, in_maps, core_ids=list(range(_NCORES)))
    return _postprocess(res.results)


if __name__ == "__main__":
    # smoke test with random data (no reference available here)
    rng = np.random.default_rng(0)
    sim = rng.standard_normal((_B, _KP + _KN), dtype=np.float32)
    pool = rng.standard_normal((_B * (1 + _KP), _P, _D), dtype=np.float32)
    idx = rng.integers(0, _B, size=(_B, _KN))
    out = kernel(similarity_matrix=sim, p_enc_out=pool, negative_index=idx)
    print("out", out.shape, out.dtype, float(np.abs(out).mean()))


# revision 10
# speedup vs baseline: 1.0386x; 1.0386x over previous
"""Trainium2 Bass kernel for nn_AggregationRebuild_HN (sparse_attention).

Computes, for each of B=512 samples:
    out[b] = sum_j softmax(sim[b] / 0.02)[j] * block_j(b)          # [64, 128]
where block_j(b) are 3 "positive" rows (512 + 3b + j of p_enc_out) and 16
gathered "negative" rows (p_enc_out[negative_index[b, j]]).

Strategy ("pruned scatter-softmax-matmul", single-pass bf16):
  * Shard the P*D = 8192 feature axis across 8 cores (1024 features each).
  * At temperature 0.02 the softmax is extremely peaked: per sample only a
    handful of the 19 candidates carry weight >= 1e-9 of the max.  The host
    keeps exactly those (sample, candidate) pairs — duplicates within a
    sample first merged in log domain (logaddexp), so the device-side exp
    reproduces the summed softmax numerators — and packs them densely into
    K-slots of 128-tall chunks (typically 2 chunks per 128-sample tile).
    Dropped slots contribute < ~1e-8 relative error; the softmax
    DENOMINATOR is still computed on device from all 19 logits, exactly.
  * Each core ships only the pool rows referenced by live slots (host
    fancy-index gather, bf16): ~2.1 MiB instead of 16.8 MiB fp32 dense.
    Input DMAs alternate between the SP and ACT HWDGE rings, which map to
    disjoint 16-engine SDMA groups, roughly doubling load bandwidth.
  * The gather + weighted sum becomes short PE-matmul chains:
    out[tile, :] accumulates C matmuls [K=128 slots, M=128 samples,
    N=512 features] per feature half, in bf16 with fp32 PSUM accumulate.
  * WT (softmax numerators scattered into slot positions) is produced on
    device by ACT Exp over a host-built fp16 scatter of the max-shifted
    logits (empty slots hold -3e4 -> exp -> 0), cast straight to bf16, in
    per-tile pieces so the first matmuls start as soon as the first piece
    lands.  1/Z lands as a per-partition scale on the PSUM->SBUF drain,
    which writes bf16 in 256-col pieces (stores pipeline on the SP ring);
    the host upcasts on unshard.  End-to-end rel err ~2e-3, inside the
    2e-2 gate.
  * Dummy matmuls before and between the real chains hold the PE HAM
    activity window busy so the clock stays at 2.4 GHz (cold/idle default
    is 1.2 GHz, which also slows DMA and the drain engines).
  * Host-side work is index bookkeeping (pruning, scatter layout,
    log-domain duplicate merge, row gather), dtype casts, and the
    standard stable-softmax max shift only; exp/sum/normalize and all
    matvec math run on device.
"""

from contextlib import ExitStack

import numpy as np

_B = 512            # bs * n_vars
_P = 64             # patch_num
_D = 128            # d_model
_KP = 3             # k_positive
_KN = 16            # k_negative
_NCORES = 8
_PPC = _P // _NCORES        # patches per core = 8
_PDC = _PPC * _D            # features per core = 1024
_SENT = -3.0e4              # empty-slot sentinel; exp(50 * -3e4) == 0
_SCALE = 50.0               # 1 / temperature
_NTILES = _B // 128         # 4 M-tiles of 128 samples
_NLOGIT = _NTILES * (_KP + _KN)   # 76 raw-logit columns (for Z)
_WTHR = -20.72              # ln(1e-9): keep slots with weight >= 1e-9*max
_NWARM = 12                 # PE warm-up dummy matmuls


def _build_host(sim, neg_idx):
    """Prune + pack slots, build scatter and row list (vectorized).

    Returns (sscw, logits, rows, C):
      C: K-chunks of 128 slots per 128-sample tile (uniform, padded)
      sscw [128, 4*128*C] f16, sentinel in empty cells:
        slot s (chunk c = s>>7, partition p = s&127) of tile t, owned by
        sample b = 128t+m, holds its merged shifted logit at
        col 128*C*t + 128*c + m
      logits [128, 76] f32: raw shifted logits, [m, t*19+k] (for Z)
      rows [4*128*C]: p_enc_out row feeding each slot (0 for pad slots)
    """
    sim = np.asarray(sim, np.float32)
    neg_idx = np.asarray(neg_idx).astype(np.int64)
    simsh = (sim - sim.max(axis=1, keepdims=True)).astype(np.float32)  # [B, 19]

    # merge duplicate negative rows per sample in log domain
    acc = np.full((_B, _B), -np.inf)
    bb = np.repeat(np.arange(_B), _KN)
    np.logaddexp.at(
        acc, (bb, neg_idx.ravel()),
        (_SCALE * simsh[:, _KP:].astype(np.float64)).ravel(),
    )
    nb, nr = np.nonzero(acc >= _WTHR)            # relevant negatives
    nv = (acc[nb, nr] / _SCALE).astype(np.float32)

    pb, pj = np.nonzero(simsh[:, :_KP] >= _WTHR / _SCALE)  # relevant positives
    pv = simsh[pb, pj]
    pr = _B + _KP * pb + pj

    samp = np.concatenate([nb, pb])
    rows = np.concatenate([nr, pr])
    vals = np.concatenate([nv, pv])
    tile = samp >> 7
    order = np.argsort(tile, kind="stable")
    samp, rows, vals, tile = samp[order], rows[order], vals[order], tile[order]
    counts = np.bincount(tile, minlength=_NTILES)
    C = max(1, int(-(-counts.max() // 128)))
    starts = np.concatenate([[0], np.cumsum(counts)[:-1]])
    slot = np.arange(len(samp)) - starts[tile]   # slot index within tile

    nslot = 128 * C
    sscw = np.full((128, _NTILES * nslot), _SENT, np.float16)
    col = nslot * tile + 128 * (slot >> 7) + (samp & 127)
    sscw[slot & 127, col] = vals.astype(np.float16)
    logits = np.ascontiguousarray(
        simsh.reshape(_NTILES, 128, 19).transpose(1, 0, 2).reshape(128, -1)
    )

    row_list = np.zeros(_NTILES * nslot, np.int64)
    row_list[nslot * tile + slot] = rows
    return sscw, logits, row_list, C


def _kernel_body(ctx, tc, out_ap, pool_ap, sscw_ap, logit_ap, C):
    import concourse.mybir as mybir

    nc = tc.nc
    f32 = mybir.dt.float32
    f16 = mybir.dt.float16
    bf16 = mybir.dt.bfloat16
    AF = mybir.ActivationFunctionType
    nslot = 128 * C
    nsloc = _NTILES * nslot            # total slot columns

    const = ctx.enter_context(tc.tile_pool(name="const", bufs=1))
    psum_pool = ctx.enter_context(tc.tile_pool(name="psum", bufs=8, space="PSUM"))

    ps = {
        (t, h): psum_pool.tile(
            [128, 512], f32, tag=f"ps{t}{h}", name=f"ps{t}{h}", bufs=1
        )
        for t in range(_NTILES)
        for h in range(2)
    }

    # --- PE warm-up + HAM keepalive: dummy matmuls into ps[3,1], which the
    # real (3,1) chain's start=True reset wipes -----------------------------
    warm = const.tile([128, 512], bf16, tag="warm")
    nc.vector.memset(warm[:], 0.0)

    def dummy_mm(n):
        # full-width (N=512) matmuls: the HAM activity window needs a HIGH
        # sustained busy fraction — short dummies fail to un-throttle
        for _ in range(n):
            nc.tensor.matmul(
                ps[3, 1][:], lhsT=warm[:, 0:128], rhs=warm[:],
                start=True, stop=True, skip_group_check=True,
            )

    dummy_mm(_NWARM)

    sscw = const.tile([128, nsloc], f16, tag="sscw")
    logit = const.tile([128, _NLOGIT], f32, tag="logit")
    whi = const.tile([128, nsloc], bf16, tag="whi")
    pool_sb = const.tile([128, _NTILES * C * _PDC], bf16, tag="pool")

    def load_tile(t):
        # pool on one HWDGE ring, scatter on the other (disjoint SDMA
        # engine groups); alternate by tile parity to balance bytes
        pool_eng = nc.sync if (t % 2 == 0) else nc.scalar
        ssc_eng = nc.scalar if (t % 2 == 0) else nc.sync
        c0 = nslot * t
        ssc_eng.dma_start(
            out=sscw[:, c0 : c0 + nslot], in_=sscw_ap[:, c0 : c0 + nslot]
        )
        view = pool_ap[nslot * t : nslot * (t + 1)].rearrange(
            "(c p) n -> c p n", p=128
        )
        o0 = C * _PDC * t
        pool_eng.dma_start(
            out=pool_sb[:, o0 : o0 + C * _PDC].rearrange("p (c n) -> p c n", n=_PDC),
            in_=view.rearrange("c p n -> p c n"),
        )

    def split_w(t):
        c0 = nslot * t
        nc.scalar.activation(
            out=whi[:, c0 : c0 + nslot], in_=sscw[:, c0 : c0 + nslot],
            func=AF.Exp, scale=_SCALE,
        )

    load_tile(0)
    nc.sync.dma_start(out=logit[:], in_=logit_ap[:])
    split_w(0)
    for t in range(1, _NTILES):
        load_tile(t)
        split_w(t)

    # softmax denominator Z[b] and 1/Z (exp + row-sum fused via accum_out)
    rz = const.tile([128, _NTILES], f32, tag="rz")
    for t in range(_NTILES):
        e = const.tile([128, 19], f32, tag=f"e{t}", name=f"e{t}")
        z = const.tile([128, 1], f32, tag=f"z{t}", name=f"z{t}")
        nc.scalar.activation(
            out=e[:],
            in_=logit[:, 19 * t : 19 * (t + 1)],
            func=AF.Exp,
            scale=_SCALE,
            accum_out=z[:],
        )
        nc.vector.reciprocal(out=rz[:, t : t + 1], in_=z[:])

    # --- matmul chains + per-tile drains (split in 256-col pieces so the
    # stores pipeline); drains alternate ACT / DVE; stores on the SP ring --
    out_sb = const.tile([128, _NTILES * _PDC], bf16, tag="out_sb")
    out_view = out_ap.rearrange("(t p) n -> t p n", p=128)
    for t in range(_NTILES):
        for h in range(2):
            for c in range(C):
                nc.tensor.matmul(
                    ps[t, h][:],
                    lhsT=whi[:, nslot * t + 128 * c : nslot * t + 128 * (c + 1)],
                    rhs=pool_sb[
                        :,
                        (C * t + c) * _PDC + 512 * h : (C * t + c) * _PDC + 512 * h + 512,
                    ],
                    start=(c == 0),
                    stop=(c == C - 1),
                    skip_group_check=True,
                )
        last = t == _NTILES - 1
        # the last tile's drains split in halves so its stores start sooner
        for h in range(2):
            for p0, p1 in ([(0, 256), (256, 512)] if last else [(0, 512)]):
                dst = out_sb[:, _PDC * t + 512 * h + p0 : _PDC * t + 512 * h + p1]
                if h == 0:
                    nc.scalar.activation(
                        out=dst, in_=ps[t, h][:, p0:p1],
                        func=AF.Copy, scale=rz[:, t : t + 1],
                    )
                    nc.scalar.dma_start(
                        out=out_view[t, :, 512 * h + p0 : 512 * h + p1], in_=dst
                    )
                else:
                    nc.vector.tensor_scalar_mul(
                        dst, ps[t, h][:, p0:p1], rz[:, t : t + 1]
                    )
                    nc.sync.dma_start(
                        out=out_view[t, :, 512 * h + p0 : 512 * h + p1], in_=dst
                    )
        if not last:
            dummy_mm(3)  # HAM keepalive while waiting for the next tile


_prog_cache = {}


def _get_program(C):
    if C in _prog_cache:
        return _prog_cache[C]
    import concourse.bacc as bacc
    import concourse.mybir as mybir
    import concourse.tile as tile

    nc = bacc.Bacc(
        "TRN2",
        target_bir_lowering=False,
        debug=False,
        enable_asserts=False,
        num_devices=_NCORES,
    )
    f32 = mybir.dt.float32
    f16 = mybir.dt.float16
    bf16 = mybir.dt.bfloat16
    pool_ap = nc.dram_tensor(
        "pool", [_NTILES * 128 * C, _PDC], bf16, kind="ExternalInput"
    ).ap()
    sscw_ap = nc.dram_tensor(
        "sscw", [128, _NTILES * 128 * C], f16, kind="ExternalInput"
    ).ap()
    logit_ap = nc.dram_tensor(
        "logit", [128, _NLOGIT], f32, kind="ExternalInput"
    ).ap()
    out_ap = nc.dram_tensor("out", [_B, _PDC], bf16, kind="ExternalOutput").ap()
    with tile.TileContext(nc) as tc:
        with ExitStack() as ctx:
            _kernel_body(ctx, tc, out_ap, pool_ap, sscw_ap, logit_ap, C)
    nc.compile()
    _prog_cache[C] = nc
    return nc


def _prepare(similarity_matrix, p_enc_out, negative_index):
    import ml_dtypes

    sim = np.asarray(similarity_matrix, np.float32)
    pool = np.asarray(p_enc_out, np.float32)
    assert sim.shape == (_B, _KP + _KN), sim.shape
    assert pool.shape == (_B * (1 + _KP), _P, _D), pool.shape
    sscw, logits, row_list, C = _build_host(sim, negative_index)
    poolb = pool.astype(ml_dtypes.bfloat16)      # [2048, 64, 128]
    gathered = poolb[row_list]                   # [4*128*C, 64, 128]
    in_maps = []
    for c in range(_NCORES):
        sl = gathered[:, _PPC * c : _PPC * (c + 1), :].reshape(-1, _PDC)
        in_maps.append(
            {"pool": np.ascontiguousarray(sl), "sscw": sscw, "logit": logits}
        )
    return in_maps, C


def _postprocess(results):
    outs = [
        r["out"].astype(np.float32).reshape(_B, _PPC, _D) for r in results
    ]
    return np.ascontiguousarray(np.concatenate(outs, axis=1))


def kernel(similarity_matrix, p_enc_out, negative_index, **_unused):
    from concourse.bass_utils import run_bass_kernel_spmd

    in_maps, C = _prepare(similarity_matrix, p_enc_out, negative_index)
    nc = _get_program(C)
    res = run_bass_kernel_spmd(nc, in_maps, core_ids=list(range(_NCORES)))
    return _postprocess(res.results)


if __name__ == "__main__":
    # smoke test with random data (no reference available here)
    rng = np.random.default_rng(0)
    sim = rng.standard_normal((_B, _KP + _KN), dtype=np.float32)
    pool = rng.standard_normal((_B * (1 + _KP), _P, _D), dtype=np.float32)
    idx = rng.integers(0, _B, size=(_B, _KN))
    out = kernel(similarity_matrix=sim, p_enc_out=pool, negative_index=idx)
    print("out", out.shape, out.dtype, float(np.abs(out).mean()))
